# revision 1
# baseline (speedup 1.0000x reference)
"""Trainium2 Bass kernel for the CustomLSTM problem.

Contract: kernel(**inputs) takes the FULL unsharded numpy inputs
(x [4096,16,512] f32, per-gate weights/biases) and returns the FULL
output h_last [4096, 1024] f32.

Strategy (data-parallel over 8 NeuronCores):
  - shard batch B=4096 -> 512 per core; replicate weights.
  - per core, per timestep t, compute fused gates in transposed layout
    gT [4H=4096, B=512] as one PSUM accumulation per 128-row gate tile:
        gT[tile] = sum_kd W[kd,tile].T @ xT_t[kd] + sum_kh U[kh,tile].T @ hT[kh]
    (K = D + H = 1536 contraction, 12 matmuls of K=128, N=512).
  - sigmoid/tanh run on ScalarE straight out of PSUM with the per-gate
    bias applied via the activation instruction's per-partition bias.
  - c stays fp32 in SBUF; h is written bf16 for the next matmul.
  - matmuls run in bf16 (fp32 PSUM accumulation).
"""

import numpy as np
import ml_dtypes

import concourse.bacc as bacc
import concourse.mybir as mybir
from concourse.tile import TileContext
from concourse.bass_utils import run_bass_kernel_spmd

F32 = mybir.dt.float32
BF16 = mybir.dt.bfloat16
AF = mybir.ActivationFunctionType

B, T, D, H = 4096, 16, 512, 1024
NCORES = 8
BL = B // NCORES          # batch per core
G = 4 * H                 # fused gate dim
KD = D // 128             # x contraction tiles
KH = H // 128             # h contraction tiles
NGT = G // 128            # gate tiles


def build_lstm(nc):
    x_d = nc.declare_dram_parameter("x", [T * D, BL], BF16, isOutput=False)
    w_d = nc.declare_dram_parameter("w", [D, G], BF16, isOutput=False)
    u_d = nc.declare_dram_parameter("u", [H, G], BF16, isOutput=False)
    b_d = nc.declare_dram_parameter("b", [128, NGT], F32, isOutput=False)
    out_d = nc.declare_dram_parameter("h_out", [H, BL], F32, isOutput=True)

    GATES = [("f", AF.Sigmoid), ("i", AF.Sigmoid), ("o", AF.Sigmoid),
             ("c", AF.Tanh)]

    with TileContext(nc) as tc:
        with tc.tile_pool(name="const", bufs=1) as cpool, \
             tc.tile_pool(name="xp", bufs=2) as xpool, \
             tc.tile_pool(name="hp", bufs=2) as hpool, \
             tc.tile_pool(name="gp", bufs=2) as gpool, \
             tc.tile_pool(name="tp", bufs=2) as tpool, \
             tc.tile_pool(name="ps", bufs=8, space="PSUM") as pspool:
            w_sb = cpool.tile([128, KD * G], BF16, name="w_sb")
            for kd in range(KD):
                nc.sync.dma_start(out=w_sb[:, kd * G:(kd + 1) * G],
                                  in_=w_d[kd * 128:(kd + 1) * 128, :])
            u_sb = cpool.tile([128, KH * G], BF16, name="u_sb")
            for kh in range(KH):
                nc.sync.dma_start(out=u_sb[:, kh * G:(kh + 1) * G],
                                  in_=u_d[kh * 128:(kh + 1) * 128, :])
            b_sb = cpool.tile([128, NGT], F32, name="b_sb")
            nc.sync.dma_start(out=b_sb[:], in_=b_d[:])
            # c state, fp32; becomes the fp32 output h at t = T-1
            c_sb = cpool.tile([128, KH * BL], F32, name="c_sb")

            h_prev = None
            for t in range(T):
                x_t = xpool.tile([128, KD * BL], BF16, name=f"x_{t}", tag="x")
                for kd in range(KD):
                    nc.sync.dma_start(
                        out=x_t[:, kd * BL:(kd + 1) * BL],
                        in_=x_d[t * D + kd * 128: t * D + (kd + 1) * 128, :])
                h_new = (hpool.tile([128, KH * BL], BF16, name=f"h_{t}", tag="h")
                         if t < T - 1 else None)
                for ht in range(KH):
                    gates = gpool.tile([128, 4 * BL], F32,
                                       name=f"gates_{t}_{ht}", tag="g")
                    for gi, (gname, func) in enumerate(GATES):
                        gt = gi * KH + ht
                        ps = pspool.tile([128, BL], F32,
                                         name=f"ps_{t}_{gt}", tag="ps")
                        nmm = KD + (KH if t > 0 else 0)
                        k = 0
                        for kd in range(KD):
                            nc.tensor.matmul(
                                ps[:],
                                w_sb[:, kd * G + gt * 128: kd * G + gt * 128 + 128],
                                x_t[:, kd * BL:(kd + 1) * BL],
                                start=(k == 0), stop=(k == nmm - 1))
                            k += 1
                        if t > 0:
                            for kh in range(KH):
                                nc.tensor.matmul(
                                    ps[:],
                                    u_sb[:, kh * G + gt * 128: kh * G + gt * 128 + 128],
                                    h_prev[:, kh * BL:(kh + 1) * BL],
                                    start=False, stop=(k == nmm - 1))
                                k += 1
                        nc.scalar.activation(
                            gates[:, gi * BL:(gi + 1) * BL], ps[:], func,
                            bias=b_sb[:, gt:gt + 1])
                    gf = gates[:, 0 * BL:1 * BL]
                    gi_ = gates[:, 1 * BL:2 * BL]
                    go = gates[:, 2 * BL:3 * BL]
                    gc = gates[:, 3 * BL:4 * BL]
                    cs = c_sb[:, ht * BL:(ht + 1) * BL]
                    tmp = tpool.tile([128, 2 * BL], F32,
                                     name=f"tmp_{t}_{ht}", tag="tmp")
                    t1 = tmp[:, 0 * BL:1 * BL]
                    t2 = tmp[:, 1 * BL:2 * BL]
                    if t == 0:
                        nc.vector.tensor_mul(cs, gi_, gc)
                    else:
                        nc.vector.tensor_mul(t1, gf, cs)
                        nc.vector.tensor_mul(t2, gi_, gc)
                        nc.vector.tensor_add(cs, t1, t2)
                    nc.scalar.activation(t1, cs, AF.Tanh)
                    if t < T - 1:
                        nc.vector.tensor_mul(
                            h_new[:, ht * BL:(ht + 1) * BL], go, t1)
                    else:
                        # final h overwrites the c slice (c dead after tanh)
                        nc.vector.tensor_mul(cs, go, t1)
                h_prev = h_new
            for kh in range(KH):
                nc.sync.dma_start(out=out_d[kh * 128:(kh + 1) * 128, :],
                                  in_=c_sb[:, kh * BL:(kh + 1) * BL])
    return nc


_BUILT = None


def _get_built():
    global _BUILT
    if _BUILT is None:
        nc = bacc.Bacc("TRN2", num_devices=NCORES)
        build_lstm(nc)
        nc.compile()
        _BUILT = nc
    return _BUILT


def _prep_inputs(x, wf, wi, wo, wc, uf, ui, uo, uc, bf, bi, bo, bc):
    bf16 = ml_dtypes.bfloat16
    W = np.concatenate([wf, wi, wo, wc], axis=1).astype(bf16)      # [D, 4H]
    U = np.concatenate([uf, ui, uo, uc], axis=1).astype(bf16)      # [H, 4H]
    b = np.concatenate([bf, bi, bo, bc], axis=1).astype(np.float32)
    b_t = np.ascontiguousarray(b.reshape(NGT, 128).T)              # [128, NGT]
    # x [B, T, D] -> per-core [T*D, BL] with xT[t*D+d, b] = x[b, t, d]
    xt = np.ascontiguousarray(np.transpose(x, (1, 2, 0)).astype(bf16))
    in_maps = []
    for c in range(NCORES):
        xc = np.ascontiguousarray(
            xt[:, :, c * BL:(c + 1) * BL].reshape(T * D, BL))
        in_maps.append({"x": xc, "w": W, "u": U, "b": b_t})
    return in_maps


def kernel(x, wf, wi, wo, wc, uf, ui, uo, uc, bf, bi, bo, bc):
    nc = _get_built()
    in_maps = _prep_inputs(x, wf, wi, wo, wc, uf, ui, uo, uc, bf, bi, bo, bc)
    res = run_bass_kernel_spmd(nc, in_maps, list(range(NCORES)))
    out = np.empty((B, H), np.float32)
    for c in range(NCORES):
        out[c * BL:(c + 1) * BL, :] = res.results[c]["h_out"].T
    return out
